# revision 28
# baseline (speedup 1.0000x reference)
"""Trainium2 Bass kernel for nn_AdditiveAttention (sine-factorized).

out[b,i,j] = softmax((masked_scores + gumbel)/tau)[..., 0]
           = sigmoid((s + gd)/tau),
  s  = sum_h wd_h * tanh(q[b,i,h] + k[b,j,h]),  wd = Wv[0]-Wv[1]
  gd = g0-g1 + 1e6*[mask==1],  g = jax.random.gumbel(key(42), (B,Lq,Lk,2))
  q  = queries @ Wq.T,  k = keys @ Wk.T

tanh(x) ~= sum_m a_m*sin(w_m*x) (free-frequency LSQ fit), and
sin(w(q+k)) = sin(wq)cos(wk) + cos(wq)sin(wk): 2M rank-1 terms -> PE matmuls
over H=128. Trig args are range-reduced on the vector engine with the fp32
magic-round trick; sin/cos via the ScalarE Sin LUT (valid |arg| <= ~pi).

Sharding: 8 cores; core c handles batch b=c//2, query rows (c%2)*256..+256.
"""
import math
import numpy as np

TAU = 0.01
B, LQ, LK, DQ, H = 4, 512, 512, 256, 128
ROWS = 256
NCORES = 8
MAGIC = 12582912.0  # 1.5 * 2**23

# Frequencies (rad per unit x) from an offline free-frequency sine fit of tanh
# on [-8.1, 8.1]. First F32M entries are the large-amplitude modes (run with
# exact-fp32 matmuls); the rest use float32r (full-rate) matmuls.
OMEGAS = [
    0.34380, 1.03383, 1.72036,                     # |a| ~ 1.24, 0.34, 0.15
    0.62240, 1.34314, 2.03784, 2.40178, 2.72366,
    3.07760, 3.39828, 3.74543, 4.04068, 4.39213,
    4.56812, 4.92142, 4.93346, 5.91081, 5.91088,
]
F32M = 3
NB = 3  # ring depth for RS/UC/SIN/COS/stat pools (f32r class)


def _fit_amplitudes(omegas, xmax, npts=8001):
    xs = np.linspace(-xmax, xmax, npts)
    wgt = np.sqrt(np.exp(-xs**2 / (2 * 1.14**2)) + 0.02)
    A = np.sin(np.outer(xs, omegas))
    coef, *_ = np.linalg.lstsq(A * wgt[:, None], np.tanh(xs) * wgt, rcond=1e-12)
    return coef


def _gumbel_diff_const():
    import jax
    import jax.numpy as jnp
    g = np.asarray(jax.random.gumbel(jax.random.key(42), (B, LQ, LK, 2),
                                     dtype=jnp.float32))
    return g[..., 0] - g[..., 1]


def _schedule(M):
    """Semaphore count tables mirroring the emission order below.
    Vector loop: iteration m emits D,RS,UC of mode m and STA/STB of mode m-2.
    Scalar: P acts two modes ahead; SIN/COS per mode; sigmoids at the end."""
    sv = 2  # 2 X copies
    v_rs, v_uc = {}, {}
    for m in range(M):
        sv += 1; v_rs[m] = sv
        sv += 1; v_uc[m] = sv
    v_add = [sv + 1, sv + 2]
    v_stb = {j: 2 * (j + 1) for j in range(M)}  # on the sGs semaphore now

    ss = 0
    s_p, s_sin, s_cos = {}, {}, {}
    for m in range(M):
        ss += 1; s_sin[m] = ss
        ss += 1; s_cos[m] = ss
    s_sig = [ss + 1, ss + 2]

    t_mm_end = {m: 4 + 4 * (m + 1) for m in range(M)}
    return v_rs, v_uc, v_stb, v_add, s_p, s_sin, s_cos, s_sig, t_mm_end


def _build_program(omegas, detect_races=True):
    import concourse.bass as bass
    import concourse.mybir as mybir
    from contextlib import ExitStack

    AF = mybir.ActivationFunctionType
    ALU = mybir.AluOpType
    f32 = mybir.dt.float32
    f32r = mybir.dt.float32r

    M = len(omegas)
    (v_rs, v_uc, v_stb, v_add, s_p, s_sin, s_cos, s_sig,
     t_mm_end) = _schedule(M)
    N_DMA_IN = 11

    def is32(m):
        return m < F32M

    nc = bass.Bass(detect_race_conditions=detect_races)
    qT_d = nc.dram_tensor("queriesT_c", [2, 128, ROWS], f32, kind="ExternalInput")
    kT_d = nc.dram_tensor("keysT_c", [2, 128, LK], f32, kind="ExternalInput")
    wqT_d = nc.dram_tensor("WqT", [2, 128, H], f32, kind="ExternalInput")
    wkT_d = nc.dram_tensor("WkT", [2, 128, H], f32, kind="ExternalInput")
    awd_d = nc.dram_tensor("awd", [H, M], f32, kind="ExternalInput")
    gd_d = nc.dram_tensor("gd_c", [ROWS, LK], f32, kind="ExternalInput")
    out_d = nc.dram_tensor("out_c", [ROWS, LK], f32, kind="ExternalOutput")

    es = ExitStack()
    sbuf = lambda name, shape, dt=f32: es.enter_context(nc.sbuf_tensor(name, shape, dt))
    psum = lambda name, shape: es.enter_context(nc.psum_tensor(name, shape, f32))

    with es:
        awd_sb = sbuf("awd_sb", [128, M])
        gd_sb = sbuf("gd_sb", [128, 2, 512])
        halfpi = sbuf("halfpi", [128, 1])
        wqT = sbuf("wqT", [128, 256])
        wkT = sbuf("wkT", [128, 256])
        qsT = sbuf("qsT", [128, 512])
        ksT = sbuf("ksT", [128, 1024])
        X = sbuf("X", [128, 768])
        Pt = [sbuf(f"Pt{i}", [128, 768]) for i in range(2)]
        Dt = [sbuf(f"Dt{i}", [128, 768]) for i in range(2)]
        RS = [sbuf(f"RS{i}", [128, 768]) for i in range(NB)]
        UC = [sbuf(f"UC{i}", [128, 768]) for i in range(NB)]
        SIN_r = [sbuf(f"SINr{i}", [128, 768], f32r) for i in range(NB)]
        COS_r = [sbuf(f"COSr{i}", [128, 768], f32r) for i in range(NB)]
        SIN_32 = [sbuf(f"SIN32{i}", [128, 768]) for i in range(2)]
        COS_32 = [sbuf(f"COS32{i}", [128, 768]) for i in range(2)]
        STA_r = [sbuf(f"STAr{i}", [128, 256], f32r) for i in range(NB)]
        STB_r = [sbuf(f"STBr{i}", [128, 256], f32r) for i in range(NB)]
        STA_32 = [sbuf(f"STA32{i}", [128, 256]) for i in range(2)]
        STB_32 = [sbuf(f"STB32{i}", [128, 256]) for i in range(2)]
        AWDB = sbuf("AWDB", [128, M * 256])  # awd col m broadcast to [128,256]
        ADD_t = [sbuf(f"ADDt{i}", [128, 512]) for i in range(2)]
        OUT_t = [sbuf(f"OUTt{i}", [128, 512]) for i in range(2)]
        acc_ps = [psum("acc0", [128, 512]), psum("acc1", [128, 512])]
        ptr_ps = [psum("ptr0", [128, 512]), psum("ptr1", [128, 512])]
        qT_ps = psum("qT_ps", [128, 512])  # full bank; only [:, 0:256] used
        kT_ps = psum("kT_ps", [128, 512])

        dma_in = es.enter_context(nc.semaphore("dma_in"))
        sI = es.enter_context(nc.semaphore("sI"))
        sG = es.enter_context(nc.semaphore("sG"))
        sGs = es.enter_context(nc.semaphore("sGs"))
        sS = es.enter_context(nc.semaphore("sS"))
        sV = es.enter_context(nc.semaphore("sV"))
        sT = es.enter_context(nc.semaphore("sT"))
        dma_out = es.enter_context(nc.semaphore("dma_out"))
        block = es.enter_context(nc.Block())

        def sin_t(m):
            return SIN_32[m % 2] if is32(m) else SIN_r[(m - F32M) % NB]

        def cos_t(m):
            return COS_32[m % 2] if is32(m) else COS_r[(m - F32M) % NB]

        def sta_t(m):
            return STA_32[m % 2] if is32(m) else STA_r[(m - F32M) % NB]

        def stb_t(m):
            return STB_32[m % 2] if is32(m) else STB_r[(m - F32M) % NB]

        def class_gate(m):
            """Mode whose matmuls must be done before mode m's SIN/COS/STA/STB
            buffers can be rewritten (same dtype-class ring), or None."""
            if is32(m):
                return m - 2 if m - 2 >= 0 else None
            return m - NB if (m - F32M) >= NB else None

        @block.gpsimd
        def _(g):
            g.wait_ge(dma_in, 16 * 8)  # wqT/wkT/qT/kT only
            g.nop().then_inc(sG, 1)
            for j in range(M):
                gcl = class_gate(j)
                if gcl is not None:
                    g.wait_ge(sT, t_mm_end[gcl])
                g.wait_ge(sS, s_cos[j])
                nc.gpsimd.tensor_tensor(sta_t(j)[:], sin_t(j)[:, 0:256],
                                        AWDB[:, j * 256:(j + 1) * 256],
                                        ALU.mult).then_inc(sGs, 1)
                nc.gpsimd.tensor_tensor(stb_t(j)[:], cos_t(j)[:, 0:256],
                                        AWDB[:, j * 256:(j + 1) * 256],
                                        ALU.mult).then_inc(sGs, 1)

        @block.tensor
        def _(t):
            t.wait_ge(sG, 1)
            for dc in range(2):
                nc.tensor.matmul(qT_ps[:, 0:256], wqT[:, dc * 128:(dc + 1) * 128],
                                 qsT[:, dc * 256:(dc + 1) * 256],
                                 start=(dc == 0), stop=(dc == 1)).then_inc(sT, 1)
            for dc in range(2):
                nc.tensor.matmul(kT_ps[:], wkT[:, dc * 128:(dc + 1) * 128],
                                 ksT[:, dc * 512:(dc + 1) * 512],
                                 start=(dc == 0), stop=(dc == 1)).then_inc(sT, 1)
            for m in range(M):
                t.wait_ge(sGs, v_stb[m])
                SINm, COSm = sin_t(m), cos_t(m)
                STAm, STBm = sta_t(m), stb_t(m)
                for ib in range(2):
                    nc.tensor.matmul(acc_ps[ib][:], STAm[:, ib * 128:(ib + 1) * 128],
                                     COSm[:, 256:768], start=(m == 0), stop=False,
                                     skip_group_check=True).then_inc(sT, 1)
                for ib in range(2):
                    nc.tensor.matmul(acc_ps[ib][:], STBm[:, ib * 128:(ib + 1) * 128],
                                     SINm[:, 256:768], start=False, stop=(m == M - 1),
                                     skip_group_check=True).then_inc(sT, 1)

        @block.vector
        def _(v):
            nc.vector.memset(halfpi[:], float(np.pi / 2))
            v.wait_ge(dma_in, 16 * 9)  # awd arrives 9th
            for m in range(M):
                nc.vector.tensor_copy(AWDB[:, m * 256:(m + 1) * 256],
                                      awd_sb[:, m:m + 1].to_broadcast((128, 256)))
            v.wait_ge(sT, 2)
            nc.vector.tensor_copy(X[:, 0:256], qT_ps[:, 0:256]).then_inc(sV, 1)
            v.wait_ge(sT, 4)
            nc.vector.tensor_copy(X[:, 256:768], kT_ps[:]).then_inc(sV, 1)

            for m in range(M):
                if m >= NB:
                    v.wait_ge(sS, s_cos[m - NB])  # RS/UC ring reuse
                sc = float(omegas[m] / (2 * math.pi))
                nc.vector.tensor_scalar(Pt[m % 2][:], X[:], sc, MAGIC,
                                        ALU.mult, ALU.add)
                nc.vector.tensor_scalar(Dt[m % 2][:], Pt[m % 2][:], -MAGIC, None,
                                        ALU.add, ALU.bypass)
                nc.vector.scalar_tensor_tensor(RS[m % NB][:], X[:], sc, Dt[m % 2][:],
                                               ALU.mult, ALU.subtract).then_inc(sV, 1)
                # |r| for the cos path: cos(2*pi*r) = sin(pi/2 - 2*pi*|r|)
                nc.vector.scalar_tensor_tensor(UC[m % NB][:], RS[m % NB][:], -1.0,
                                               RS[m % NB][:], ALU.mult,
                                               ALU.max).then_inc(sV, 1)
            v.wait_ge(dma_in, 16 * N_DMA_IN)
            v.wait_ge(sT, t_mm_end[M - 1])
            for ib in range(2):
                nc.vector.tensor_tensor(ADD_t[ib][:], acc_ps[ib][:], gd_sb[:, ib, :],
                                        ALU.add).then_inc(sV, 1)

        @block.scalar
        def _(sc_eng):
            for m in range(M):
                g = class_gate(m)
                if g is not None:
                    sc_eng.wait_ge(sT, t_mm_end[g])
                sc_eng.wait_ge(sV, v_rs[m])
                nc.scalar.activation(sin_t(m)[:], RS[m % NB][:], AF.Sin,
                                     scale=float(2 * math.pi)).then_inc(sS, 1)
                sc_eng.wait_ge(sV, v_uc[m])
                nc.scalar.activation(cos_t(m)[:], UC[m % NB][:], AF.Sin,
                                     scale=float(-2 * math.pi),
                                     bias=halfpi[:]).then_inc(sS, 1)
            for ib in range(2):
                sc_eng.wait_ge(sV, v_add[ib])
                nc.scalar.activation(OUT_t[ib][:], ADD_t[ib][:], AF.Sigmoid,
                                     scale=float(1.0 / TAU)).then_inc(sS, 1)

        @block.sync
        def _(s):
            s.dma_start(wqT[:, 0:128], wqT_d[0]).then_inc(dma_in, 16)
            s.dma_start(wqT[:, 128:256], wqT_d[1]).then_inc(dma_in, 16)
            s.dma_start(wkT[:, 0:128], wkT_d[0]).then_inc(dma_in, 16)
            s.dma_start(wkT[:, 128:256], wkT_d[1]).then_inc(dma_in, 16)
            s.dma_start(qsT[:, 0:256], qT_d[0]).then_inc(dma_in, 16)
            s.dma_start(qsT[:, 256:512], qT_d[1]).then_inc(dma_in, 16)
            s.dma_start(ksT[:, 0:512], kT_d[0]).then_inc(dma_in, 16)
            s.dma_start(ksT[:, 512:1024], kT_d[1]).then_inc(dma_in, 16)
            s.dma_start(awd_sb[:], awd_d[:]).then_inc(dma_in, 16)
            for ib in range(2):
                s.dma_start(gd_sb[:, ib, :],
                            gd_d[ib * 128:(ib + 1) * 128, :]).then_inc(dma_in, 16)
            for ib in range(2):
                s.wait_ge(sS, s_sig[ib])
                s.dma_start(out_d[ib * 128:(ib + 1) * 128, :],
                            OUT_t[ib][:]).then_inc(dma_out, 16)
            s.wait_ge(dma_out, 32)

    return nc


_NC_CACHE = {}


def kernel(queries, keys, mask, Wq, Wk, Wv):
    from concourse.bass_utils import run_bass_kernel_spmd

    queries = np.ascontiguousarray(queries, dtype=np.float32)
    keys = np.ascontiguousarray(keys, dtype=np.float32)
    Wq = np.ascontiguousarray(Wq, dtype=np.float32)
    Wk = np.ascontiguousarray(Wk, dtype=np.float32)
    Wv = np.ascontiguousarray(Wv, dtype=np.float32)
    mask = np.asarray(mask)

    wd = (Wv[0] - Wv[1]).astype(np.float32)
    gd = (_gumbel_diff_const()
          + np.where(mask == 1, np.float32(1e6), np.float32(0.0))).astype(np.float32)

    q = queries @ Wq.T
    k = keys @ Wk.T
    xmax = float(max((q.max(axis=1) + k.max(axis=1)).max(),
                     -((q.min(axis=1) + k.min(axis=1)).min()))) + 0.05
    omegas = np.asarray(OMEGAS, dtype=np.float64)
    coef = _fit_amplitudes(omegas, max(xmax, 6.0))
    awd = (coef[None, :].astype(np.float32) * wd[:, None]).astype(np.float32)

    if "nc" not in _NC_CACHE:
        _NC_CACHE["nc"] = _build_program(OMEGAS)
    nc = _NC_CACHE["nc"]

    wqT_h = np.ascontiguousarray(Wq.T.reshape(2, 128, H))
    wkT_h = np.ascontiguousarray(Wk.T.reshape(2, 128, H))
    in_maps = []
    for c in range(NCORES):
        b, ih = c // 2, c % 2
        qT = np.ascontiguousarray(
            queries[b, ih * ROWS:(ih + 1) * ROWS, :].T.reshape(2, 128, ROWS))
        kT = np.ascontiguousarray(keys[b].T.reshape(2, 128, LK))
        in_maps.append({
            "queriesT_c": qT,
            "keysT_c": kT,
            "WqT": wqT_h, "WkT": wkT_h, "awd": awd,
            "gd_c": np.ascontiguousarray(gd[b, ih * ROWS:(ih + 1) * ROWS, :]),
        })

    res = run_bass_kernel_spmd(nc, in_maps, list(range(NCORES)))
    out = np.empty((B, LQ, LK), dtype=np.float32)
    for c in range(NCORES):
        b, ih = c // 2, c % 2
        out[b, ih * ROWS:(ih + 1) * ROWS, :] = res.results[c]["out_c"]
    return out


# revision 29
# speedup vs baseline: 1.1320x; 1.1320x over previous
"""Trainium2 Bass kernel for nn_AdditiveAttention (sine-factorized).

out[b,i,j] = softmax((masked_scores + gumbel)/tau)[..., 0]
           = sigmoid((s + gd)/tau),
  s  = sum_h wd_h * tanh(q[b,i,h] + k[b,j,h]),  wd = Wv[0]-Wv[1]
  gd = g0-g1 + 1e6*[mask==1],  g = jax.random.gumbel(key(42), (B,Lq,Lk,2))
  q  = queries @ Wq.T,  k = keys @ Wk.T

tanh(x) ~= sum_m a_m*sin(w_m*x) (free-frequency LSQ fit), and
sin(w(q+k)) = sin(wq)cos(wk) + cos(wq)sin(wk): 2M rank-1 terms -> PE matmuls
over H=128. Trig args are range-reduced on the vector engine with the fp32
magic-round trick; sin/cos via the ScalarE Sin LUT (valid |arg| <= ~pi).

Sharding: 8 cores; core c handles batch b=c//2, query rows (c%2)*256..+256.
"""
import math
import numpy as np

TAU = 0.01
B, LQ, LK, DQ, H = 4, 512, 512, 256, 128
ROWS = 256
NCORES = 8
MAGIC = 12582912.0  # 1.5 * 2**23

# Frequencies (rad per unit x) from an offline free-frequency sine fit of tanh
# on [-8.1, 8.1]. First F32M entries are the large-amplitude modes (run with
# exact-fp32 matmuls); the rest use float32r (full-rate) matmuls.
OMEGAS = [
    0.34380, 1.03383, 1.72036,                     # |a| ~ 1.24, 0.34, 0.15
    0.62240, 1.34314, 2.03784, 2.40178, 2.72366,
    3.07760, 3.39828, 3.74543, 4.04068, 4.39213,
    4.56812, 4.92142, 4.93346, 5.91081, 5.91088,
]
F32M = 3
NB = 3  # ring depth for RS/UC/SIN/COS/stat pools (f32r class)


def _fit_amplitudes(omegas, xmax, npts=8001):
    xs = np.linspace(-xmax, xmax, npts)
    wgt = np.sqrt(np.exp(-xs**2 / (2 * 1.14**2)) + 0.02)
    A = np.sin(np.outer(xs, omegas))
    coef, *_ = np.linalg.lstsq(A * wgt[:, None], np.tanh(xs) * wgt, rcond=1e-12)
    return coef


def _gumbel_diff_const():
    import jax
    import jax.numpy as jnp
    g = np.asarray(jax.random.gumbel(jax.random.key(42), (B, LQ, LK, 2),
                                     dtype=jnp.float32))
    return g[..., 0] - g[..., 1]


def _schedule(M):
    """Semaphore count tables mirroring the emission order below.
    Vector loop: iteration m emits D,RS,UC of mode m and STA/STB of mode m-2.
    Scalar: P acts two modes ahead; SIN/COS per mode; sigmoids at the end."""
    sv = 2  # 2 X copies
    v_rs, v_uc, v_sta, v_stb = {}, {}, {}, {}
    for m in range(M):
        sv += 1; v_rs[m] = sv
        sv += 1; v_uc[m] = sv
        j = m - 2
        if j >= 0:
            sv += 1; v_sta[j] = sv
            sv += 1; v_stb[j] = sv
    for j in (M - 2, M - 1):
        sv += 1; v_sta[j] = sv
        sv += 1; v_stb[j] = sv
    v_add = [sv + 1, sv + 2]

    ss = 0
    s_p, s_sin, s_cos = {}, {}, {}
    for m in range(M):
        ss += 1; s_sin[m] = ss
        ss += 1; s_cos[m] = ss
    s_sig = [ss + 1, ss + 2]

    t_mm_end = {m: 4 + 4 * (m + 1) for m in range(M)}
    return v_rs, v_uc, v_stb, v_add, s_p, s_sin, s_cos, s_sig, t_mm_end


def _build_program(omegas, detect_races=True):
    import concourse.bass as bass
    import concourse.mybir as mybir
    from contextlib import ExitStack

    AF = mybir.ActivationFunctionType
    ALU = mybir.AluOpType
    f32 = mybir.dt.float32
    f32r = mybir.dt.float32r

    M = len(omegas)
    (v_rs, v_uc, v_stb, v_add, s_p, s_sin, s_cos, s_sig,
     t_mm_end) = _schedule(M)
    N_DMA_IN = 11

    def is32(m):
        return m < F32M

    nc = bass.Bass(detect_race_conditions=detect_races)
    qT_d = nc.dram_tensor("queriesT_c", [2, 128, ROWS], f32, kind="ExternalInput")
    kT_d = nc.dram_tensor("keysT_c", [2, 128, LK], f32, kind="ExternalInput")
    wqT_d = nc.dram_tensor("WqT", [2, 128, H], f32, kind="ExternalInput")
    wkT_d = nc.dram_tensor("WkT", [2, 128, H], f32, kind="ExternalInput")
    awd_d = nc.dram_tensor("awd", [H, M], f32, kind="ExternalInput")
    gd_d = nc.dram_tensor("gd_c", [ROWS, LK], f32, kind="ExternalInput")
    out_d = nc.dram_tensor("out_c", [ROWS, LK], f32, kind="ExternalOutput")

    es = ExitStack()
    sbuf = lambda name, shape, dt=f32: es.enter_context(nc.sbuf_tensor(name, shape, dt))
    psum = lambda name, shape: es.enter_context(nc.psum_tensor(name, shape, f32))

    with es:
        awd_sb = sbuf("awd_sb", [128, M])
        gd_sb = sbuf("gd_sb", [128, 2, 512])
        halfpi = sbuf("halfpi", [128, 1])
        wqT = sbuf("wqT", [128, 256])
        wkT = sbuf("wkT", [128, 256])
        qsT = sbuf("qsT", [128, 512])
        ksT = sbuf("ksT", [128, 1024])
        X = sbuf("X", [128, 768])
        Pt = [sbuf(f"Pt{i}", [128, 768]) for i in range(2)]
        Dt = [sbuf(f"Dt{i}", [128, 768]) for i in range(2)]
        RS = [sbuf(f"RS{i}", [128, 768]) for i in range(NB)]
        UC = [sbuf(f"UC{i}", [128, 768]) for i in range(NB)]
        SIN_r = [sbuf(f"SINr{i}", [128, 768], f32r) for i in range(NB)]
        COS_r = [sbuf(f"COSr{i}", [128, 768], f32r) for i in range(NB)]
        SIN_32 = [sbuf(f"SIN32{i}", [128, 768]) for i in range(2)]
        COS_32 = [sbuf(f"COS32{i}", [128, 768]) for i in range(2)]
        STA_r = [sbuf(f"STAr{i}", [128, 256], f32r) for i in range(NB)]
        STB_r = [sbuf(f"STBr{i}", [128, 256], f32r) for i in range(NB)]
        STA_32 = [sbuf(f"STA32{i}", [128, 256]) for i in range(2)]
        STB_32 = [sbuf(f"STB32{i}", [128, 256]) for i in range(2)]
        AWDB = sbuf("AWDB", [128, M * 256])  # awd col m broadcast to [128,256]
        ADD_t = [sbuf(f"ADDt{i}", [128, 512]) for i in range(2)]
        OUT_t = [sbuf(f"OUTt{i}", [128, 512]) for i in range(2)]
        acc_ps = [psum("acc0", [128, 512]), psum("acc1", [128, 512])]
        ptr_ps = [psum("ptr0", [128, 512]), psum("ptr1", [128, 512])]
        qT_ps = psum("qT_ps", [128, 512])  # full bank; only [:, 0:256] used
        kT_ps = psum("kT_ps", [128, 512])

        dma_in = es.enter_context(nc.semaphore("dma_in"))
        sI = es.enter_context(nc.semaphore("sI"))
        sG = es.enter_context(nc.semaphore("sG"))
        sS = es.enter_context(nc.semaphore("sS"))
        sV = es.enter_context(nc.semaphore("sV"))
        sT = es.enter_context(nc.semaphore("sT"))
        dma_out = es.enter_context(nc.semaphore("dma_out"))
        block = es.enter_context(nc.Block())

        def sin_t(m):
            return SIN_32[m % 2] if is32(m) else SIN_r[(m - F32M) % NB]

        def cos_t(m):
            return COS_32[m % 2] if is32(m) else COS_r[(m - F32M) % NB]

        def sta_t(m):
            return STA_32[m % 2] if is32(m) else STA_r[(m - F32M) % NB]

        def stb_t(m):
            return STB_32[m % 2] if is32(m) else STB_r[(m - F32M) % NB]

        def class_gate(m):
            """Mode whose matmuls must be done before mode m's SIN/COS/STA/STB
            buffers can be rewritten (same dtype-class ring), or None."""
            if is32(m):
                return m - 2 if m - 2 >= 0 else None
            return m - NB if (m - F32M) >= NB else None

        @block.gpsimd
        def _(g):
            g.wait_ge(dma_in, 16 * 8)  # wqT/wkT/qT/kT only
            g.nop().then_inc(sG, 1)

        @block.tensor
        def _(t):
            t.wait_ge(sG, 1)
            for dc in range(2):
                nc.tensor.matmul(qT_ps[:, 0:256], wqT[:, dc * 128:(dc + 1) * 128],
                                 qsT[:, dc * 256:(dc + 1) * 256],
                                 start=(dc == 0), stop=(dc == 1)).then_inc(sT, 1)
            for dc in range(2):
                nc.tensor.matmul(kT_ps[:], wkT[:, dc * 128:(dc + 1) * 128],
                                 ksT[:, dc * 512:(dc + 1) * 512],
                                 start=(dc == 0), stop=(dc == 1)).then_inc(sT, 1)
            for m in range(M):
                t.wait_ge(sV, v_stb[m])
                SINm, COSm = sin_t(m), cos_t(m)
                STAm, STBm = sta_t(m), stb_t(m)
                for ib in range(2):
                    nc.tensor.matmul(acc_ps[ib][:], STAm[:, ib * 128:(ib + 1) * 128],
                                     COSm[:, 256:768], start=(m == 0), stop=False,
                                     skip_group_check=True).then_inc(sT, 1)
                for ib in range(2):
                    nc.tensor.matmul(acc_ps[ib][:], STBm[:, ib * 128:(ib + 1) * 128],
                                     SINm[:, 256:768], start=False, stop=(m == M - 1),
                                     skip_group_check=True).then_inc(sT, 1)

        @block.vector
        def _(v):
            nc.vector.memset(halfpi[:], float(np.pi / 2))
            v.wait_ge(dma_in, 16 * 9)  # awd arrives 9th
            for m in range(M):
                nc.vector.tensor_copy(AWDB[:, m * 256:(m + 1) * 256],
                                      awd_sb[:, m:m + 1].to_broadcast((128, 256)))
            v.wait_ge(sT, 2)
            nc.vector.tensor_copy(X[:, 0:256], qT_ps[:, 0:256]).then_inc(sV, 1)
            v.wait_ge(sT, 4)
            nc.vector.tensor_copy(X[:, 256:768], kT_ps[:]).then_inc(sV, 1)

            def stat_ops(j):
                g = class_gate(j)
                if g is not None:
                    v.wait_ge(sT, t_mm_end[g])
                v.wait_ge(sS, s_cos[j])
                nc.vector.tensor_tensor(sta_t(j)[:], sin_t(j)[:, 0:256],
                                        AWDB[:, j * 256:(j + 1) * 256],
                                        ALU.mult).then_inc(sV, 1)
                nc.vector.tensor_tensor(stb_t(j)[:], cos_t(j)[:, 0:256],
                                        AWDB[:, j * 256:(j + 1) * 256],
                                        ALU.mult).then_inc(sV, 1)

            for m in range(M):
                if m >= NB:
                    v.wait_ge(sS, s_cos[m - NB])  # RS/UC ring reuse
                sc = float(omegas[m] / (2 * math.pi))
                nc.vector.tensor_scalar(Pt[m % 2][:], X[:], sc, MAGIC,
                                        ALU.mult, ALU.add)
                nc.vector.tensor_scalar(Dt[m % 2][:], Pt[m % 2][:], -MAGIC, None,
                                        ALU.add, ALU.bypass)
                nc.vector.scalar_tensor_tensor(RS[m % NB][:], X[:], sc, Dt[m % 2][:],
                                               ALU.mult, ALU.subtract).then_inc(sV, 1)
                # |r| for the cos path: cos(2*pi*r) = sin(pi/2 - 2*pi*|r|)
                nc.vector.scalar_tensor_tensor(UC[m % NB][:], RS[m % NB][:], -1.0,
                                               RS[m % NB][:], ALU.mult,
                                               ALU.max).then_inc(sV, 1)
                if m >= 2:
                    stat_ops(m - 2)
            stat_ops(M - 2)
            stat_ops(M - 1)
            v.wait_ge(dma_in, 16 * N_DMA_IN)
            v.wait_ge(sT, t_mm_end[M - 1])
            for ib in range(2):
                nc.vector.tensor_tensor(ADD_t[ib][:], acc_ps[ib][:], gd_sb[:, ib, :],
                                        ALU.add).then_inc(sV, 1)

        @block.scalar
        def _(sc_eng):
            for m in range(M):
                g = class_gate(m)
                if g is not None:
                    sc_eng.wait_ge(sT, t_mm_end[g])
                sc_eng.wait_ge(sV, v_rs[m])
                nc.scalar.activation(sin_t(m)[:], RS[m % NB][:], AF.Sin,
                                     scale=float(2 * math.pi)).then_inc(sS, 1)
                sc_eng.wait_ge(sV, v_uc[m])
                nc.scalar.activation(cos_t(m)[:], UC[m % NB][:], AF.Sin,
                                     scale=float(-2 * math.pi),
                                     bias=halfpi[:]).then_inc(sS, 1)
            for ib in range(2):
                sc_eng.wait_ge(sV, v_add[ib])
                nc.scalar.activation(OUT_t[ib][:], ADD_t[ib][:], AF.Sigmoid,
                                     scale=float(1.0 / TAU)).then_inc(sS, 1)

        @block.sync
        def _(s):
            s.dma_start(wqT[:, 0:128], wqT_d[0]).then_inc(dma_in, 16)
            s.dma_start(wqT[:, 128:256], wqT_d[1]).then_inc(dma_in, 16)
            s.dma_start(wkT[:, 0:128], wkT_d[0]).then_inc(dma_in, 16)
            s.dma_start(wkT[:, 128:256], wkT_d[1]).then_inc(dma_in, 16)
            s.dma_start(qsT[:, 0:256], qT_d[0]).then_inc(dma_in, 16)
            s.dma_start(qsT[:, 256:512], qT_d[1]).then_inc(dma_in, 16)
            s.dma_start(ksT[:, 0:512], kT_d[0]).then_inc(dma_in, 16)
            s.dma_start(ksT[:, 512:1024], kT_d[1]).then_inc(dma_in, 16)
            s.dma_start(awd_sb[:], awd_d[:]).then_inc(dma_in, 16)
            for ib in range(2):
                s.dma_start(gd_sb[:, ib, :],
                            gd_d[ib * 128:(ib + 1) * 128, :]).then_inc(dma_in, 16)
            for ib in range(2):
                s.wait_ge(sS, s_sig[ib])
                s.dma_start(out_d[ib * 128:(ib + 1) * 128, :],
                            OUT_t[ib][:]).then_inc(dma_out, 16)
            s.wait_ge(dma_out, 32)

    return nc


_NC_CACHE = {}


def kernel(queries, keys, mask, Wq, Wk, Wv):
    from concourse.bass_utils import run_bass_kernel_spmd

    queries = np.ascontiguousarray(queries, dtype=np.float32)
    keys = np.ascontiguousarray(keys, dtype=np.float32)
    Wq = np.ascontiguousarray(Wq, dtype=np.float32)
    Wk = np.ascontiguousarray(Wk, dtype=np.float32)
    Wv = np.ascontiguousarray(Wv, dtype=np.float32)
    mask = np.asarray(mask)

    wd = (Wv[0] - Wv[1]).astype(np.float32)
    gd = (_gumbel_diff_const()
          + np.where(mask == 1, np.float32(1e6), np.float32(0.0))).astype(np.float32)

    q = queries @ Wq.T
    k = keys @ Wk.T
    xmax = float(max((q.max(axis=1) + k.max(axis=1)).max(),
                     -((q.min(axis=1) + k.min(axis=1)).min()))) + 0.05
    omegas = np.asarray(OMEGAS, dtype=np.float64)
    coef = _fit_amplitudes(omegas, max(xmax, 6.0))
    awd = (coef[None, :].astype(np.float32) * wd[:, None]).astype(np.float32)

    if "nc" not in _NC_CACHE:
        _NC_CACHE["nc"] = _build_program(OMEGAS)
    nc = _NC_CACHE["nc"]

    wqT_h = np.ascontiguousarray(Wq.T.reshape(2, 128, H))
    wkT_h = np.ascontiguousarray(Wk.T.reshape(2, 128, H))
    in_maps = []
    for c in range(NCORES):
        b, ih = c // 2, c % 2
        qT = np.ascontiguousarray(
            queries[b, ih * ROWS:(ih + 1) * ROWS, :].T.reshape(2, 128, ROWS))
        kT = np.ascontiguousarray(keys[b].T.reshape(2, 128, LK))
        in_maps.append({
            "queriesT_c": qT,
            "keysT_c": kT,
            "WqT": wqT_h, "WkT": wkT_h, "awd": awd,
            "gd_c": np.ascontiguousarray(gd[b, ih * ROWS:(ih + 1) * ROWS, :]),
        })

    res = run_bass_kernel_spmd(nc, in_maps, list(range(NCORES)))
    out = np.empty((B, LQ, LK), dtype=np.float32)
    for c in range(NCORES):
        b, ih = c // 2, c % 2
        out[b, ih * ROWS:(ih + 1) * ROWS, :] = res.results[c]["out_c"]
    return out


# revision 30
# speedup vs baseline: 1.1488x; 1.0149x over previous
"""Trainium2 Bass kernel for nn_AdditiveAttention (sine-factorized).

out[b,i,j] = softmax((masked_scores + gumbel)/tau)[..., 0]
           = sigmoid((s + gd)/tau),
  s  = sum_h wd_h * tanh(q[b,i,h] + k[b,j,h]),  wd = Wv[0]-Wv[1]
  gd = g0-g1 + 1e6*[mask==1],  g = jax.random.gumbel(key(42), (B,Lq,Lk,2))
  q  = queries @ Wq.T,  k = keys @ Wk.T

tanh(x) ~= sum_m a_m*sin(w_m*x) (free-frequency LSQ fit), and
sin(w(q+k)) = sin(wq)cos(wk) + cos(wq)sin(wk): 2M rank-1 terms -> PE matmuls
over H=128. Trig args are range-reduced on the vector engine with the fp32
magic-round trick; sin/cos via the ScalarE Sin LUT (valid |arg| <= ~pi).

Sharding: 8 cores; core c handles batch b=c//2, query rows (c%2)*256..+256.
"""
import math
import numpy as np

TAU = 0.01
B, LQ, LK, DQ, H = 4, 512, 512, 256, 128
ROWS = 256
NCORES = 8
MAGIC = 12582912.0  # 1.5 * 2**23

# Frequencies (rad per unit x) from an offline free-frequency sine fit of tanh
# on [-8.1, 8.1]. First F32M entries are the large-amplitude modes (run with
# exact-fp32 matmuls); the rest use float32r (full-rate) matmuls.
OMEGAS = [
    0.34380, 1.03383, 1.72036,                     # |a| ~ 1.24, 0.34, 0.15
    0.62240, 1.34314, 2.03784, 2.40178, 2.72366,
    3.07760, 3.39828, 3.74543, 4.04068, 4.39213,
    4.56812, 4.92142, 4.93346, 5.91081, 5.91088,
]
F32M = 3
NB = 3  # ring depth for RS/UC/SIN/COS/stat pools (f32r class)


def _fit_amplitudes(omegas, xmax, npts=8001):
    xs = np.linspace(-xmax, xmax, npts)
    wgt = np.sqrt(np.exp(-xs**2 / (2 * 1.14**2)) + 0.02)
    A = np.sin(np.outer(xs, omegas))
    coef, *_ = np.linalg.lstsq(A * wgt[:, None], np.tanh(xs) * wgt, rcond=1e-12)
    return coef


def _gumbel_diff_const():
    import jax
    import jax.numpy as jnp
    g = np.asarray(jax.random.gumbel(jax.random.key(42), (B, LQ, LK, 2),
                                     dtype=jnp.float32))
    return g[..., 0] - g[..., 1]


def _schedule(M):
    """Semaphore count tables mirroring the emission order below.
    Vector loop: iteration m emits D,RS,UC of mode m and STA/STB of mode m-2.
    Scalar: P acts two modes ahead; SIN/COS per mode; sigmoids at the end."""
    sv = 2  # 2 X copies
    v_rs, v_uc, v_sta, v_stb = {}, {}, {}, {}
    for m in range(M):
        sv += 1; v_rs[m] = sv
        sv += 1; v_uc[m] = sv
        j = m - 2
        if j >= 0:
            sv += 1; v_sta[j] = sv
            sv += 1; v_stb[j] = sv
    for j in (M - 2, M - 1):
        sv += 1; v_sta[j] = sv
        sv += 1; v_stb[j] = sv
    v_add = [sv + 1, sv + 2]

    ss = 0
    s_p, s_sin, s_cos = {}, {}, {}
    for j in (0, 1):
        if j <= M - 1:
            ss += 1; s_p[j] = ss
    for m in range(M):
        ss += 1; s_sin[m] = ss
        ss += 1; s_cos[m] = ss
        if m + 2 <= M - 1:
            ss += 1; s_p[m + 2] = ss
    s_sig = [ss + 1, ss + 2]

    t_mm_end = {m: 4 + 4 * (m + 1) for m in range(M)}
    return v_rs, v_uc, v_stb, v_add, s_p, s_sin, s_cos, s_sig, t_mm_end


def _build_program(omegas, detect_races=True):
    import concourse.bass as bass
    import concourse.mybir as mybir
    from contextlib import ExitStack

    AF = mybir.ActivationFunctionType
    ALU = mybir.AluOpType
    f32 = mybir.dt.float32
    f32r = mybir.dt.float32r

    M = len(omegas)
    (v_rs, v_uc, v_stb, v_add, s_p, s_sin, s_cos, s_sig,
     t_mm_end) = _schedule(M)
    N_DMA_IN = 11

    def is32(m):
        return m < F32M

    nc = bass.Bass(detect_race_conditions=detect_races)
    qT_d = nc.dram_tensor("queriesT_c", [2, 128, ROWS], f32, kind="ExternalInput")
    kT_d = nc.dram_tensor("keysT_c", [2, 128, LK], f32, kind="ExternalInput")
    wqT_d = nc.dram_tensor("WqT", [2, 128, H], f32, kind="ExternalInput")
    wkT_d = nc.dram_tensor("WkT", [2, 128, H], f32, kind="ExternalInput")
    awd_d = nc.dram_tensor("awd", [H, M], f32, kind="ExternalInput")
    gd_d = nc.dram_tensor("gd_c", [ROWS, LK], f32, kind="ExternalInput")
    out_d = nc.dram_tensor("out_c", [ROWS, LK], f32, kind="ExternalOutput")

    es = ExitStack()
    sbuf = lambda name, shape, dt=f32: es.enter_context(nc.sbuf_tensor(name, shape, dt))
    psum = lambda name, shape: es.enter_context(nc.psum_tensor(name, shape, f32))

    with es:
        awd_sb = sbuf("awd_sb", [128, M])
        gd_sb = sbuf("gd_sb", [128, 2, 512])
        halfpi = sbuf("halfpi", [128, 1])
        magicb = sbuf("magicb", [128, 1])
        wqT = sbuf("wqT", [128, 256])
        wkT = sbuf("wkT", [128, 256])
        qsT = sbuf("qsT", [128, 512])
        ksT = sbuf("ksT", [128, 1024])
        X = sbuf("X", [128, 768])
        Pt = [sbuf(f"Pt{i}", [128, 768]) for i in range(2)]
        Dt = [sbuf(f"Dt{i}", [128, 768]) for i in range(2)]
        RS = [sbuf(f"RS{i}", [128, 768]) for i in range(NB)]
        UC = [sbuf(f"UC{i}", [128, 768]) for i in range(NB)]
        SIN_r = [sbuf(f"SINr{i}", [128, 768], f32r) for i in range(NB)]
        COS_r = [sbuf(f"COSr{i}", [128, 768], f32r) for i in range(NB)]
        SIN_32 = [sbuf(f"SIN32{i}", [128, 768]) for i in range(2)]
        COS_32 = [sbuf(f"COS32{i}", [128, 768]) for i in range(2)]
        STA_r = [sbuf(f"STAr{i}", [128, 256], f32r) for i in range(NB)]
        STB_r = [sbuf(f"STBr{i}", [128, 256], f32r) for i in range(NB)]
        STA_32 = [sbuf(f"STA32{i}", [128, 256]) for i in range(2)]
        STB_32 = [sbuf(f"STB32{i}", [128, 256]) for i in range(2)]
        AWDB = sbuf("AWDB", [128, M * 256])  # awd col m broadcast to [128,256]
        ADD_t = [sbuf(f"ADDt{i}", [128, 512]) for i in range(2)]
        OUT_t = [sbuf(f"OUTt{i}", [128, 512]) for i in range(2)]
        acc_ps = [psum("acc0", [128, 512]), psum("acc1", [128, 512])]
        ptr_ps = [psum("ptr0", [128, 512]), psum("ptr1", [128, 512])]
        qT_ps = psum("qT_ps", [128, 512])  # full bank; only [:, 0:256] used
        kT_ps = psum("kT_ps", [128, 512])

        dma_in = es.enter_context(nc.semaphore("dma_in"))
        sI = es.enter_context(nc.semaphore("sI"))
        sG = es.enter_context(nc.semaphore("sG"))
        sS = es.enter_context(nc.semaphore("sS"))
        sV = es.enter_context(nc.semaphore("sV"))
        sT = es.enter_context(nc.semaphore("sT"))
        dma_out = es.enter_context(nc.semaphore("dma_out"))
        block = es.enter_context(nc.Block())

        def sin_t(m):
            return SIN_32[m % 2] if is32(m) else SIN_r[(m - F32M) % NB]

        def cos_t(m):
            return COS_32[m % 2] if is32(m) else COS_r[(m - F32M) % NB]

        def sta_t(m):
            return STA_32[m % 2] if is32(m) else STA_r[(m - F32M) % NB]

        def stb_t(m):
            return STB_32[m % 2] if is32(m) else STB_r[(m - F32M) % NB]

        def class_gate(m):
            """Mode whose matmuls must be done before mode m's SIN/COS/STA/STB
            buffers can be rewritten (same dtype-class ring), or None."""
            if is32(m):
                return m - 2 if m - 2 >= 0 else None
            return m - NB if (m - F32M) >= NB else None

        @block.gpsimd
        def _(g):
            g.wait_ge(dma_in, 16 * 8)  # wqT/wkT/qT/kT only
            g.nop().then_inc(sG, 1)

        @block.tensor
        def _(t):
            t.wait_ge(sG, 1)
            for dc in range(2):
                nc.tensor.matmul(qT_ps[:, 0:256], wqT[:, dc * 128:(dc + 1) * 128],
                                 qsT[:, dc * 256:(dc + 1) * 256],
                                 start=(dc == 0), stop=(dc == 1)).then_inc(sT, 1)
            for dc in range(2):
                nc.tensor.matmul(kT_ps[:], wkT[:, dc * 128:(dc + 1) * 128],
                                 ksT[:, dc * 512:(dc + 1) * 512],
                                 start=(dc == 0), stop=(dc == 1)).then_inc(sT, 1)
            for m in range(M):
                t.wait_ge(sV, v_stb[m])
                SINm, COSm = sin_t(m), cos_t(m)
                STAm, STBm = sta_t(m), stb_t(m)
                ib_order = ((0, 0), (0, 1), (1, 0), (1, 1)) if m < M - 1 else \
                           ((0, 0), (1, 0), (0, 1), (1, 1))  # finish bank0 first
                for term, ib in ib_order:
                    STm = STAm if term == 0 else STBm
                    MVm = COSm if term == 0 else SINm
                    nc.tensor.matmul(acc_ps[ib][:], STm[:, ib * 128:(ib + 1) * 128],
                                     MVm[:, 256:768], start=(m == 0 and term == 0),
                                     stop=(m == M - 1 and term == 1),
                                     skip_group_check=True).then_inc(sT, 1)

        @block.vector
        def _(v):
            nc.vector.memset(halfpi[:], float(np.pi / 2))
            nc.vector.memset(magicb[:], MAGIC)
            v.wait_ge(dma_in, 16 * 9)  # awd arrives 9th
            for m in range(M):
                nc.vector.tensor_copy(AWDB[:, m * 256:(m + 1) * 256],
                                      awd_sb[:, m:m + 1].to_broadcast((128, 256)))
            v.wait_ge(sT, 2)
            nc.vector.tensor_copy(X[:, 0:256], qT_ps[:, 0:256]).then_inc(sV, 1)
            v.wait_ge(sT, 4)
            nc.vector.tensor_copy(X[:, 256:768], kT_ps[:]).then_inc(sV, 1)

            def stat_ops(j):
                g = class_gate(j)
                if g is not None:
                    v.wait_ge(sT, t_mm_end[g])
                v.wait_ge(sS, s_cos[j])
                nc.vector.tensor_tensor(sta_t(j)[:], sin_t(j)[:, 0:256],
                                        AWDB[:, j * 256:(j + 1) * 256],
                                        ALU.mult).then_inc(sV, 1)
                nc.vector.tensor_tensor(stb_t(j)[:], cos_t(j)[:, 0:256],
                                        AWDB[:, j * 256:(j + 1) * 256],
                                        ALU.mult).then_inc(sV, 1)

            for m in range(M):
                if m >= NB:
                    v.wait_ge(sS, s_cos[m - NB])  # RS/UC ring reuse
                sc = float(omegas[m] / (2 * math.pi))
                v.wait_ge(sS, s_p[m])
                nc.vector.tensor_scalar(Dt[m % 2][:], Pt[m % 2][:], -MAGIC, None,
                                        ALU.add, ALU.bypass)
                nc.vector.scalar_tensor_tensor(RS[m % NB][:], X[:], sc, Dt[m % 2][:],
                                               ALU.mult, ALU.subtract).then_inc(sV, 1)
                # |r| for the cos path: cos(2*pi*r) = sin(pi/2 - 2*pi*|r|)
                nc.vector.scalar_tensor_tensor(UC[m % NB][:], RS[m % NB][:], -1.0,
                                               RS[m % NB][:], ALU.mult,
                                               ALU.max).then_inc(sV, 1)
                if m >= 2:
                    stat_ops(m - 2)
            stat_ops(M - 2)
            stat_ops(M - 1)
            v.wait_ge(dma_in, 16 * N_DMA_IN)
            for ib in range(2):
                v.wait_ge(sT, t_mm_end[M - 1] - 2 + 2 * ib)
                nc.vector.tensor_tensor(ADD_t[ib][:], acc_ps[ib][:], gd_sb[:, ib, :],
                                        ALU.add).then_inc(sV, 1)

        @block.scalar
        def _(sc_eng):
            def p_act(j):
                nc.scalar.activation(Pt[j % 2][:], X[:], AF.Relu,
                                     scale=float(omegas[j] / (2 * math.pi)),
                                     bias=magicb[:]).then_inc(sS, 1)

            sc_eng.wait_ge(sV, 2)  # X ready
            p_act(0)
            if M > 1:
                p_act(1)
            for m in range(M):
                g = class_gate(m)
                if g is not None:
                    sc_eng.wait_ge(sT, t_mm_end[g])
                sc_eng.wait_ge(sV, v_rs[m])
                nc.scalar.activation(sin_t(m)[:], RS[m % NB][:], AF.Sin,
                                     scale=float(2 * math.pi)).then_inc(sS, 1)
                sc_eng.wait_ge(sV, v_uc[m])
                nc.scalar.activation(cos_t(m)[:], UC[m % NB][:], AF.Sin,
                                     scale=float(-2 * math.pi),
                                     bias=halfpi[:]).then_inc(sS, 1)
                if m + 2 <= M - 1:
                    p_act(m + 2)
            for ib in range(2):
                sc_eng.wait_ge(sV, v_add[ib])
                nc.scalar.activation(OUT_t[ib][:], ADD_t[ib][:], AF.Sigmoid,
                                     scale=float(1.0 / TAU)).then_inc(sS, 1)

        @block.sync
        def _(s):
            s.dma_start(wqT[:, 0:128], wqT_d[0]).then_inc(dma_in, 16)
            s.dma_start(wqT[:, 128:256], wqT_d[1]).then_inc(dma_in, 16)
            s.dma_start(wkT[:, 0:128], wkT_d[0]).then_inc(dma_in, 16)
            s.dma_start(wkT[:, 128:256], wkT_d[1]).then_inc(dma_in, 16)
            s.dma_start(qsT[:, 0:256], qT_d[0]).then_inc(dma_in, 16)
            s.dma_start(qsT[:, 256:512], qT_d[1]).then_inc(dma_in, 16)
            s.dma_start(ksT[:, 0:512], kT_d[0]).then_inc(dma_in, 16)
            s.dma_start(ksT[:, 512:1024], kT_d[1]).then_inc(dma_in, 16)
            s.dma_start(awd_sb[:], awd_d[:]).then_inc(dma_in, 16)
            for ib in range(2):
                s.dma_start(gd_sb[:, ib, :],
                            gd_d[ib * 128:(ib + 1) * 128, :]).then_inc(dma_in, 16)
            for ib in range(2):
                s.wait_ge(sS, s_sig[ib])
                s.dma_start(out_d[ib * 128:(ib + 1) * 128, :],
                            OUT_t[ib][:]).then_inc(dma_out, 16)
            s.wait_ge(dma_out, 32)

    return nc


_NC_CACHE = {}


def kernel(queries, keys, mask, Wq, Wk, Wv):
    from concourse.bass_utils import run_bass_kernel_spmd

    queries = np.ascontiguousarray(queries, dtype=np.float32)
    keys = np.ascontiguousarray(keys, dtype=np.float32)
    Wq = np.ascontiguousarray(Wq, dtype=np.float32)
    Wk = np.ascontiguousarray(Wk, dtype=np.float32)
    Wv = np.ascontiguousarray(Wv, dtype=np.float32)
    mask = np.asarray(mask)

    wd = (Wv[0] - Wv[1]).astype(np.float32)
    gd = (_gumbel_diff_const()
          + np.where(mask == 1, np.float32(1e6), np.float32(0.0))).astype(np.float32)

    q = queries @ Wq.T
    k = keys @ Wk.T
    xmax = float(max((q.max(axis=1) + k.max(axis=1)).max(),
                     -((q.min(axis=1) + k.min(axis=1)).min()))) + 0.05
    omegas = np.asarray(OMEGAS, dtype=np.float64)
    coef = _fit_amplitudes(omegas, max(xmax, 6.0))
    awd = (coef[None, :].astype(np.float32) * wd[:, None]).astype(np.float32)

    if "nc" not in _NC_CACHE:
        _NC_CACHE["nc"] = _build_program(OMEGAS)
    nc = _NC_CACHE["nc"]

    wqT_h = np.ascontiguousarray(Wq.T.reshape(2, 128, H))
    wkT_h = np.ascontiguousarray(Wk.T.reshape(2, 128, H))
    in_maps = []
    for c in range(NCORES):
        b, ih = c // 2, c % 2
        qT = np.ascontiguousarray(
            queries[b, ih * ROWS:(ih + 1) * ROWS, :].T.reshape(2, 128, ROWS))
        kT = np.ascontiguousarray(keys[b].T.reshape(2, 128, LK))
        in_maps.append({
            "queriesT_c": qT,
            "keysT_c": kT,
            "WqT": wqT_h, "WkT": wkT_h, "awd": awd,
            "gd_c": np.ascontiguousarray(gd[b, ih * ROWS:(ih + 1) * ROWS, :]),
        })

    res = run_bass_kernel_spmd(nc, in_maps, list(range(NCORES)))
    out = np.empty((B, LQ, LK), dtype=np.float32)
    for c in range(NCORES):
        b, ih = c // 2, c % 2
        out[b, ih * ROWS:(ih + 1) * ROWS, :] = res.results[c]["out_c"]
    return out


# revision 31
# speedup vs baseline: 1.2139x; 1.0567x over previous
"""Trainium2 Bass kernel for nn_AdditiveAttention (sine-factorized).

out[b,i,j] = softmax((masked_scores + gumbel)/tau)[..., 0]
           = sigmoid((s + gd)/tau),
  s  = sum_h wd_h * tanh(q[b,i,h] + k[b,j,h]),  wd = Wv[0]-Wv[1]
  gd = g0-g1 + 1e6*[mask==1],  g = jax.random.gumbel(key(42), (B,Lq,Lk,2))
  q  = queries @ Wq.T,  k = keys @ Wk.T

tanh(x) ~= sum_m a_m*sin(w_m*x) (free-frequency LSQ fit), and
sin(w(q+k)) = sin(wq)cos(wk) + cos(wq)sin(wk): 2M rank-1 terms -> PE matmuls
over H=128. Trig args are range-reduced on the vector engine with the fp32
magic-round trick; sin/cos via the ScalarE Sin LUT (valid |arg| <= ~pi).

Sharding: 8 cores; core c handles batch b=c//2, query rows (c%2)*256..+256.
"""
import math
import numpy as np

TAU = 0.01
B, LQ, LK, DQ, H = 4, 512, 512, 256, 128
ROWS = 256
NCORES = 8
MAGIC = 12582912.0  # 1.5 * 2**23

# Frequencies (rad per unit x) from an offline free-frequency sine fit of tanh
# on [-8.1, 8.1]. First F32M entries are the large-amplitude modes (run with
# exact-fp32 matmuls); the rest use float32r (full-rate) matmuls.
OMEGAS = [
    0.34380, 1.03383, 1.72036,                     # |a| ~ 1.24, 0.34, 0.15
    0.62240, 1.34314, 2.03784, 2.40178, 2.72366,
    3.07760, 3.39828, 3.74543, 4.04068, 4.39213,
    4.56812, 4.92142, 4.93346, 5.91081, 5.91088,
]
F32M = 3
NB = 3  # ring depth for RS/UC/SIN/COS/stat pools (f32r class)


def _fit_amplitudes(omegas, xmax, npts=8001):
    xs = np.linspace(-xmax, xmax, npts)
    wgt = np.sqrt(np.exp(-xs**2 / (2 * 1.14**2)) + 0.02)
    A = np.sin(np.outer(xs, omegas))
    coef, *_ = np.linalg.lstsq(A * wgt[:, None], np.tanh(xs) * wgt, rcond=1e-12)
    return coef


def _gumbel_diff_const():
    import jax
    import jax.numpy as jnp
    g = np.asarray(jax.random.gumbel(jax.random.key(42), (B, LQ, LK, 2),
                                     dtype=jnp.float32))
    return g[..., 0] - g[..., 1]


def _schedule(M):
    """Semaphore count tables mirroring the emission order below.
    Vector loop: iteration m emits D,RS,UC of mode m and STA/STB of mode m-2.
    Scalar: P acts two modes ahead; SIN/COS per mode; sigmoids at the end."""
    sv = 2  # 2 X copies
    v_rs, v_uc, v_sta, v_stb = {}, {}, {}, {}
    for m in range(M):
        sv += 1; v_rs[m] = sv
        sv += 1; v_uc[m] = sv
        j = m - 2
        if j >= 0:
            sv += 1; v_sta[j] = sv
            sv += 1; v_stb[j] = sv
    for j in (M - 2, M - 1):
        sv += 1; v_sta[j] = sv
        sv += 1; v_stb[j] = sv
    v_add = [sv + 1, sv + 2]

    ss = 0
    s_p, s_sin, s_cos = {}, {}, {}
    for j in (0, 1, 2, 3):
        if j <= M - 1:
            ss += 1; s_p[j] = ss
    for m in range(M):
        ss += 1; s_sin[m] = ss
        ss += 1; s_cos[m] = ss
        if m + 4 <= M - 1:
            ss += 1; s_p[m + 4] = ss
    s_sig = [ss + 1, ss + 2]

    t_mm_end = {m: 4 + 4 * (m + 1) for m in range(M)}
    return v_rs, v_uc, v_stb, v_add, s_p, s_sin, s_cos, s_sig, t_mm_end


def _build_program(omegas, detect_races=True):
    import concourse.bass as bass
    import concourse.mybir as mybir
    from contextlib import ExitStack

    AF = mybir.ActivationFunctionType
    ALU = mybir.AluOpType
    f32 = mybir.dt.float32
    f32r = mybir.dt.float32r

    M = len(omegas)
    (v_rs, v_uc, v_stb, v_add, s_p, s_sin, s_cos, s_sig,
     t_mm_end) = _schedule(M)
    N_DMA_IN = 11

    def is32(m):
        return m < F32M

    nc = bass.Bass(detect_race_conditions=detect_races)
    qT_d = nc.dram_tensor("queriesT_c", [2, 128, ROWS], f32, kind="ExternalInput")
    kT_d = nc.dram_tensor("keysT_c", [2, 128, LK], f32, kind="ExternalInput")
    wqT_d = nc.dram_tensor("WqT", [2, 128, H], f32, kind="ExternalInput")
    wkT_d = nc.dram_tensor("WkT", [2, 128, H], f32, kind="ExternalInput")
    awd_d = nc.dram_tensor("awd", [H, M], f32, kind="ExternalInput")
    gd_d = nc.dram_tensor("gd_c", [ROWS, LK], f32, kind="ExternalInput")
    out_d = nc.dram_tensor("out_c", [ROWS, LK], f32, kind="ExternalOutput")

    es = ExitStack()
    sbuf = lambda name, shape, dt=f32: es.enter_context(nc.sbuf_tensor(name, shape, dt))
    psum = lambda name, shape: es.enter_context(nc.psum_tensor(name, shape, f32))

    with es:
        awd_sb = sbuf("awd_sb", [128, M])
        gd_sb = sbuf("gd_sb", [128, 2, 512])
        halfpi = sbuf("halfpi", [128, 1])
        magicb = sbuf("magicb", [128, 1])
        wqT = sbuf("wqT", [128, 256])
        wkT = sbuf("wkT", [128, 256])
        qsT = sbuf("qsT", [128, 512])
        ksT = sbuf("ksT", [128, 1024])
        X = sbuf("X", [128, 768])
        Pt = [sbuf(f"Pt{i}", [128, 768]) for i in range(4)]
        Dt = [sbuf(f"Dt{i}", [128, 768]) for i in range(2)]
        RS = [sbuf(f"RS{i}", [128, 768]) for i in range(NB)]
        UC = [sbuf(f"UC{i}", [128, 768]) for i in range(NB)]
        SIN_r = [sbuf(f"SINr{i}", [128, 768], f32r) for i in range(NB)]
        COS_r = [sbuf(f"COSr{i}", [128, 768], f32r) for i in range(NB)]
        SIN_32 = [sbuf(f"SIN32{i}", [128, 768]) for i in range(2)]
        COS_32 = [sbuf(f"COS32{i}", [128, 768]) for i in range(2)]
        STA_r = [sbuf(f"STAr{i}", [128, 256], f32r) for i in range(NB)]
        STB_r = [sbuf(f"STBr{i}", [128, 256], f32r) for i in range(NB)]
        STA_32 = [sbuf(f"STA32{i}", [128, 256]) for i in range(2)]
        STB_32 = [sbuf(f"STB32{i}", [128, 256]) for i in range(2)]
        AWDB = sbuf("AWDB", [128, M * 256])  # awd col m broadcast to [128,256]
        ADD_t = [sbuf(f"ADDt{i}", [128, 512]) for i in range(2)]
        OUT_t = [sbuf(f"OUTt{i}", [128, 512]) for i in range(2)]
        acc_ps = [psum("acc0", [128, 512]), psum("acc1", [128, 512])]
        ptr_ps = [psum("ptr0", [128, 512]), psum("ptr1", [128, 512])]
        qT_ps = psum("qT_ps", [128, 512])  # full bank; only [:, 0:256] used
        kT_ps = psum("kT_ps", [128, 512])

        dma_in = es.enter_context(nc.semaphore("dma_in"))
        sI = es.enter_context(nc.semaphore("sI"))
        sG = es.enter_context(nc.semaphore("sG"))
        sS = es.enter_context(nc.semaphore("sS"))
        sV = es.enter_context(nc.semaphore("sV"))
        sT = es.enter_context(nc.semaphore("sT"))
        dma_out = es.enter_context(nc.semaphore("dma_out"))
        block = es.enter_context(nc.Block())

        def sin_t(m):
            return SIN_32[m % 2] if is32(m) else SIN_r[(m - F32M) % NB]

        def cos_t(m):
            return COS_32[m % 2] if is32(m) else COS_r[(m - F32M) % NB]

        def sta_t(m):
            return STA_32[m % 2] if is32(m) else STA_r[(m - F32M) % NB]

        def stb_t(m):
            return STB_32[m % 2] if is32(m) else STB_r[(m - F32M) % NB]

        def class_gate(m):
            """Mode whose matmuls must be done before mode m's SIN/COS/STA/STB
            buffers can be rewritten (same dtype-class ring), or None."""
            if is32(m):
                return m - 2 if m - 2 >= 0 else None
            return m - NB if (m - F32M) >= NB else None

        @block.gpsimd
        def _(g):
            g.wait_ge(dma_in, 16 * 8)  # wqT/wkT/qT/kT only
            g.nop().then_inc(sG, 1)

        @block.tensor
        def _(t):
            t.wait_ge(sG, 1)
            for dc in range(2):
                nc.tensor.matmul(qT_ps[:, 0:256], wqT[:, dc * 128:(dc + 1) * 128],
                                 qsT[:, dc * 256:(dc + 1) * 256],
                                 start=(dc == 0), stop=(dc == 1)).then_inc(sT, 1)
            for dc in range(2):
                nc.tensor.matmul(kT_ps[:], wkT[:, dc * 128:(dc + 1) * 128],
                                 ksT[:, dc * 512:(dc + 1) * 512],
                                 start=(dc == 0), stop=(dc == 1)).then_inc(sT, 1)
            for m in range(M):
                t.wait_ge(sV, v_stb[m])
                SINm, COSm = sin_t(m), cos_t(m)
                STAm, STBm = sta_t(m), stb_t(m)
                ib_order = ((0, 0), (0, 1), (1, 0), (1, 1)) if m < M - 1 else \
                           ((0, 0), (1, 0), (0, 1), (1, 1))  # finish bank0 first
                for term, ib in ib_order:
                    STm = STAm if term == 0 else STBm
                    MVm = COSm if term == 0 else SINm
                    nc.tensor.matmul(acc_ps[ib][:], STm[:, ib * 128:(ib + 1) * 128],
                                     MVm[:, 256:768], start=(m == 0 and term == 0),
                                     stop=(m == M - 1 and term == 1),
                                     skip_group_check=True).then_inc(sT, 1)

        @block.vector
        def _(v):
            nc.vector.memset(halfpi[:], float(np.pi / 2))
            nc.vector.memset(magicb[:], MAGIC)
            v.wait_ge(dma_in, 16 * 9)  # awd arrives 9th
            for m in range(M):
                nc.vector.tensor_copy(AWDB[:, m * 256:(m + 1) * 256],
                                      awd_sb[:, m:m + 1].to_broadcast((128, 256)))
            v.wait_ge(sT, 2)
            nc.vector.tensor_copy(X[:, 0:256], qT_ps[:, 0:256]).then_inc(sV, 1)
            v.wait_ge(sT, 4)
            nc.vector.tensor_copy(X[:, 256:768], kT_ps[:]).then_inc(sV, 1)

            def stat_ops(j):
                g = class_gate(j)
                if g is not None:
                    v.wait_ge(sT, t_mm_end[g])
                v.wait_ge(sS, s_cos[j])
                nc.vector.tensor_tensor(sta_t(j)[:], sin_t(j)[:, 0:256],
                                        AWDB[:, j * 256:(j + 1) * 256],
                                        ALU.mult).then_inc(sV, 1)
                nc.vector.tensor_tensor(stb_t(j)[:], cos_t(j)[:, 0:256],
                                        AWDB[:, j * 256:(j + 1) * 256],
                                        ALU.mult).then_inc(sV, 1)

            for m in range(M):
                if m >= NB:
                    v.wait_ge(sS, s_cos[m - NB])  # RS/UC ring reuse
                sc = float(omegas[m] / (2 * math.pi))
                v.wait_ge(sS, s_p[m])
                nc.vector.tensor_scalar(Dt[m % 2][:], Pt[m % 4][:], -MAGIC, None,
                                        ALU.add, ALU.bypass)
                nc.vector.scalar_tensor_tensor(RS[m % NB][:], X[:], sc, Dt[m % 2][:],
                                               ALU.mult, ALU.subtract).then_inc(sV, 1)
                # |r| for the cos path: cos(2*pi*r) = sin(pi/2 - 2*pi*|r|)
                nc.vector.scalar_tensor_tensor(UC[m % NB][:], RS[m % NB][:], -1.0,
                                               RS[m % NB][:], ALU.mult,
                                               ALU.max).then_inc(sV, 1)
                if m >= 2:
                    stat_ops(m - 2)
            stat_ops(M - 2)
            stat_ops(M - 1)
            v.wait_ge(dma_in, 16 * N_DMA_IN)
            for ib in range(2):
                v.wait_ge(sT, t_mm_end[M - 1] - 2 + 2 * ib)
                nc.vector.tensor_tensor(ADD_t[ib][:], acc_ps[ib][:], gd_sb[:, ib, :],
                                        ALU.add).then_inc(sV, 1)

        @block.scalar
        def _(sc_eng):
            def p_act(j):
                nc.scalar.activation(Pt[j % 4][:], X[:], AF.Relu,
                                     scale=float(omegas[j] / (2 * math.pi)),
                                     bias=magicb[:]).then_inc(sS, 1)

            sc_eng.wait_ge(sV, 2)  # X ready
            for j in range(min(4, M)):
                p_act(j)
            for m in range(M):
                g = class_gate(m)
                if g is not None:
                    sc_eng.wait_ge(sT, t_mm_end[g])
                sc_eng.wait_ge(sV, v_rs[m])
                nc.scalar.activation(sin_t(m)[:], RS[m % NB][:], AF.Sin,
                                     scale=float(2 * math.pi)).then_inc(sS, 1)
                sc_eng.wait_ge(sV, v_uc[m])
                nc.scalar.activation(cos_t(m)[:], UC[m % NB][:], AF.Sin,
                                     scale=float(-2 * math.pi),
                                     bias=halfpi[:]).then_inc(sS, 1)
                if m + 4 <= M - 1:
                    p_act(m + 4)
            for ib in range(2):
                sc_eng.wait_ge(sV, v_add[ib])
                nc.scalar.activation(OUT_t[ib][:], ADD_t[ib][:], AF.Sigmoid,
                                     scale=float(1.0 / TAU)).then_inc(sS, 1)

        @block.sync
        def _(s):
            s.dma_start(wqT[:, 0:128], wqT_d[0]).then_inc(dma_in, 16)
            s.dma_start(wqT[:, 128:256], wqT_d[1]).then_inc(dma_in, 16)
            s.dma_start(wkT[:, 0:128], wkT_d[0]).then_inc(dma_in, 16)
            s.dma_start(wkT[:, 128:256], wkT_d[1]).then_inc(dma_in, 16)
            s.dma_start(qsT[:, 0:256], qT_d[0]).then_inc(dma_in, 16)
            s.dma_start(qsT[:, 256:512], qT_d[1]).then_inc(dma_in, 16)
            s.dma_start(ksT[:, 0:512], kT_d[0]).then_inc(dma_in, 16)
            s.dma_start(ksT[:, 512:1024], kT_d[1]).then_inc(dma_in, 16)
            s.dma_start(awd_sb[:], awd_d[:]).then_inc(dma_in, 16)
            for ib in range(2):
                s.dma_start(gd_sb[:, ib, :],
                            gd_d[ib * 128:(ib + 1) * 128, :]).then_inc(dma_in, 16)
            for ib in range(2):
                s.wait_ge(sS, s_sig[ib])
                s.dma_start(out_d[ib * 128:(ib + 1) * 128, :],
                            OUT_t[ib][:]).then_inc(dma_out, 16)
            s.wait_ge(dma_out, 32)

    return nc


_NC_CACHE = {}


def kernel(queries, keys, mask, Wq, Wk, Wv):
    from concourse.bass_utils import run_bass_kernel_spmd

    queries = np.ascontiguousarray(queries, dtype=np.float32)
    keys = np.ascontiguousarray(keys, dtype=np.float32)
    Wq = np.ascontiguousarray(Wq, dtype=np.float32)
    Wk = np.ascontiguousarray(Wk, dtype=np.float32)
    Wv = np.ascontiguousarray(Wv, dtype=np.float32)
    mask = np.asarray(mask)

    wd = (Wv[0] - Wv[1]).astype(np.float32)
    gd = (_gumbel_diff_const()
          + np.where(mask == 1, np.float32(1e6), np.float32(0.0))).astype(np.float32)

    q = queries @ Wq.T
    k = keys @ Wk.T
    xmax = float(max((q.max(axis=1) + k.max(axis=1)).max(),
                     -((q.min(axis=1) + k.min(axis=1)).min()))) + 0.05
    omegas = np.asarray(OMEGAS, dtype=np.float64)
    coef = _fit_amplitudes(omegas, max(xmax, 6.0))
    awd = (coef[None, :].astype(np.float32) * wd[:, None]).astype(np.float32)

    if "nc" not in _NC_CACHE:
        _NC_CACHE["nc"] = _build_program(OMEGAS)
    nc = _NC_CACHE["nc"]

    wqT_h = np.ascontiguousarray(Wq.T.reshape(2, 128, H))
    wkT_h = np.ascontiguousarray(Wk.T.reshape(2, 128, H))
    in_maps = []
    for c in range(NCORES):
        b, ih = c // 2, c % 2
        qT = np.ascontiguousarray(
            queries[b, ih * ROWS:(ih + 1) * ROWS, :].T.reshape(2, 128, ROWS))
        kT = np.ascontiguousarray(keys[b].T.reshape(2, 128, LK))
        in_maps.append({
            "queriesT_c": qT,
            "keysT_c": kT,
            "WqT": wqT_h, "WkT": wkT_h, "awd": awd,
            "gd_c": np.ascontiguousarray(gd[b, ih * ROWS:(ih + 1) * ROWS, :]),
        })

    res = run_bass_kernel_spmd(nc, in_maps, list(range(NCORES)))
    out = np.empty((B, LQ, LK), dtype=np.float32)
    for c in range(NCORES):
        b, ih = c // 2, c % 2
        out[b, ih * ROWS:(ih + 1) * ROWS, :] = res.results[c]["out_c"]
    return out


# revision 33
# speedup vs baseline: 1.2437x; 1.0246x over previous
"""Trainium2 Bass kernel for nn_AdditiveAttention (sine-factorized).

out[b,i,j] = softmax((masked_scores + gumbel)/tau)[..., 0]
           = sigmoid((s + gd)/tau),
  s  = sum_h wd_h * tanh(q[b,i,h] + k[b,j,h]),  wd = Wv[0]-Wv[1]
  gd = g0-g1 + 1e6*[mask==1],  g = jax.random.gumbel(key(42), (B,Lq,Lk,2))
  q  = queries @ Wq.T,  k = keys @ Wk.T

tanh(x) ~= sum_m a_m*sin(w_m*x) (free-frequency LSQ fit), and
sin(w(q+k)) = sin(wq)cos(wk) + cos(wq)sin(wk): 2M rank-1 terms -> PE matmuls
over H=128. Trig args are range-reduced on the vector engine with the fp32
magic-round trick; sin/cos via the ScalarE Sin LUT (valid |arg| <= ~pi).

Sharding: 8 cores; core c handles batch b=c//2, query rows (c%2)*256..+256.
"""
import math
import numpy as np

TAU = 0.01
B, LQ, LK, DQ, H = 4, 512, 512, 256, 128
ROWS = 256
NCORES = 8
MAGIC = 12582912.0  # 1.5 * 2**23

# Frequencies (rad per unit x) from an offline free-frequency sine fit of tanh
# on [-8.1, 8.1]. First F32M entries are the large-amplitude modes (run with
# exact-fp32 matmuls); the rest use float32r (full-rate) matmuls.
OMEGAS = [
    0.34380, 1.03383, 1.72036,                     # |a| ~ 1.24, 0.34, 0.15
    0.62240, 1.34314, 2.03784, 2.40178, 2.72366,
    3.07760, 3.39828, 3.74543, 4.04068, 4.39213,
    4.56812, 4.92142, 4.93346, 5.91081, 5.91088,
]
F32M = 3
NB = 4  # ring depth for RS/UC/SIN/COS/stat pools (f32r class)


def _fit_amplitudes(omegas, xmax, npts=8001):
    xs = np.linspace(-xmax, xmax, npts)
    wgt = np.sqrt(np.exp(-xs**2 / (2 * 1.14**2)) + 0.02)
    A = np.sin(np.outer(xs, omegas))
    coef, *_ = np.linalg.lstsq(A * wgt[:, None], np.tanh(xs) * wgt, rcond=1e-12)
    return coef


def _gumbel_diff_const():
    import jax
    import jax.numpy as jnp
    g = np.asarray(jax.random.gumbel(jax.random.key(42), (B, LQ, LK, 2),
                                     dtype=jnp.float32))
    return g[..., 0] - g[..., 1]


def _schedule(M):
    """Semaphore count tables mirroring the emission order below.
    Vector loop: iteration m emits D,RS,UC of mode m and STA/STB of mode m-2.
    Scalar: P acts two modes ahead; SIN/COS per mode; sigmoids at the end."""
    sv = 2  # 2 X copies
    v_rs, v_uc, v_sta, v_stb = {}, {}, {}, {}
    for m in range(M):
        sv += 1; v_rs[m] = sv
        sv += 1; v_uc[m] = sv
        j = m - 2
        if j >= 0:
            sv += 1; v_sta[j] = sv
            sv += 1; v_stb[j] = sv
    for j in (M - 2, M - 1):
        sv += 1; v_sta[j] = sv
        sv += 1; v_stb[j] = sv
    v_add = [sv + 1, sv + 2]

    ss = 0
    s_p, s_sin, s_cos = {}, {}, {}
    for j in (0, 1, 2, 3):
        if j <= M - 1:
            ss += 1; s_p[j] = ss
    for m in range(M):
        ss += 1; s_sin[m] = ss
        ss += 1; s_cos[m] = ss
        if m + 4 <= M - 1:
            ss += 1; s_p[m + 4] = ss
    s_sig = [ss + 1, ss + 2]

    t_mm_end = {m: 4 + 4 * (m + 1) for m in range(M)}
    return v_rs, v_uc, v_stb, v_add, s_p, s_sin, s_cos, s_sig, t_mm_end


def _build_program(omegas, detect_races=True):
    import concourse.bass as bass
    import concourse.mybir as mybir
    from contextlib import ExitStack

    AF = mybir.ActivationFunctionType
    ALU = mybir.AluOpType
    f32 = mybir.dt.float32
    f32r = mybir.dt.float32r

    M = len(omegas)
    (v_rs, v_uc, v_stb, v_add, s_p, s_sin, s_cos, s_sig,
     t_mm_end) = _schedule(M)
    N_DMA_IN = 11

    def is32(m):
        return m < F32M

    nc = bass.Bass(detect_race_conditions=detect_races)
    qT_d = nc.dram_tensor("queriesT_c", [2, 128, ROWS], f32, kind="ExternalInput")
    kT_d = nc.dram_tensor("keysT_c", [2, 128, LK], f32, kind="ExternalInput")
    wqT_d = nc.dram_tensor("WqT", [2, 128, H], f32, kind="ExternalInput")
    wkT_d = nc.dram_tensor("WkT", [2, 128, H], f32, kind="ExternalInput")
    awd_d = nc.dram_tensor("awd", [H, M], f32, kind="ExternalInput")
    gd_d = nc.dram_tensor("gd_c", [ROWS, LK], f32, kind="ExternalInput")
    out_d = nc.dram_tensor("out_c", [ROWS, LK], f32, kind="ExternalOutput")

    es = ExitStack()
    sbuf = lambda name, shape, dt=f32: es.enter_context(nc.sbuf_tensor(name, shape, dt))
    psum = lambda name, shape: es.enter_context(nc.psum_tensor(name, shape, f32))

    with es:
        awd_sb = sbuf("awd_sb", [128, M])
        gd_sb = sbuf("gd_sb", [128, 2, 512])
        halfpi = sbuf("halfpi", [128, 1])
        magicb = sbuf("magicb", [128, 1])
        wqT = sbuf("wqT", [128, 256])
        wkT = sbuf("wkT", [128, 256])
        qsT = sbuf("qsT", [128, 512])
        ksT = sbuf("ksT", [128, 1024])
        X = sbuf("X", [128, 768])
        Pt = [sbuf(f"Pt{i}", [128, 768]) for i in range(4)]
        Dt = [sbuf(f"Dt{i}", [128, 768]) for i in range(2)]
        RS = [sbuf(f"RS{i}", [128, 768]) for i in range(NB)]
        UC = [sbuf(f"UC{i}", [128, 768]) for i in range(NB)]
        SIN_r = [sbuf(f"SINr{i}", [128, 768], f32r) for i in range(NB)]
        COS_r = [sbuf(f"COSr{i}", [128, 768], f32r) for i in range(NB)]
        SIN_32 = [sbuf(f"SIN32{i}", [128, 768]) for i in range(3)]
        COS_32 = [sbuf(f"COS32{i}", [128, 768]) for i in range(3)]
        STA_r = [sbuf(f"STAr{i}", [128, 256], f32r) for i in range(NB)]
        STB_r = [sbuf(f"STBr{i}", [128, 256], f32r) for i in range(NB)]
        STA_32 = [sbuf(f"STA32{i}", [128, 256]) for i in range(3)]
        STB_32 = [sbuf(f"STB32{i}", [128, 256]) for i in range(3)]
        AWDB = sbuf("AWDB", [128, M * 256])  # awd col m broadcast to [128,256]
        ADD_t = [sbuf(f"ADDt{i}", [128, 512]) for i in range(2)]
        OUT_t = [sbuf(f"OUTt{i}", [128, 512]) for i in range(2)]
        acc_ps = [psum("acc0", [128, 512]), psum("acc1", [128, 512])]
        ptr_ps = [psum("ptr0", [128, 512]), psum("ptr1", [128, 512])]
        qT_ps = psum("qT_ps", [128, 512])  # full bank; only [:, 0:256] used
        kT_ps = psum("kT_ps", [128, 512])

        dma_in = es.enter_context(nc.semaphore("dma_in"))
        sI = es.enter_context(nc.semaphore("sI"))
        sG = es.enter_context(nc.semaphore("sG"))
        sS = es.enter_context(nc.semaphore("sS"))
        sV = es.enter_context(nc.semaphore("sV"))
        sT = es.enter_context(nc.semaphore("sT"))
        dma_out = es.enter_context(nc.semaphore("dma_out"))
        block = es.enter_context(nc.Block())

        def sin_t(m):
            return SIN_32[m % 3] if is32(m) else SIN_r[(m - F32M) % NB]

        def cos_t(m):
            return COS_32[m % 3] if is32(m) else COS_r[(m - F32M) % NB]

        def sta_t(m):
            return STA_32[m % 3] if is32(m) else STA_r[(m - F32M) % NB]

        def stb_t(m):
            return STB_32[m % 3] if is32(m) else STB_r[(m - F32M) % NB]

        def class_gate(m):
            """Mode whose matmuls must be done before mode m's SIN/COS/STA/STB
            buffers can be rewritten (same dtype-class ring), or None."""
            if is32(m):
                return None  # 3 f32 modes, 3 buffers: no reuse
            return m - NB if (m - F32M) >= NB else None

        @block.gpsimd
        def _(g):
            g.wait_ge(dma_in, 16 * 8)  # wqT/wkT/qT/kT only
            g.nop().then_inc(sG, 1)

        @block.tensor
        def _(t):
            t.wait_ge(sG, 1)
            for dc in range(2):
                nc.tensor.matmul(qT_ps[:, 0:256], wqT[:, dc * 128:(dc + 1) * 128],
                                 qsT[:, dc * 256:(dc + 1) * 256],
                                 start=(dc == 0), stop=(dc == 1)).then_inc(sT, 1)
            for dc in range(2):
                nc.tensor.matmul(kT_ps[:], wkT[:, dc * 128:(dc + 1) * 128],
                                 ksT[:, dc * 512:(dc + 1) * 512],
                                 start=(dc == 0), stop=(dc == 1)).then_inc(sT, 1)
            for m in range(M):
                t.wait_ge(sV, v_stb[m])
                SINm, COSm = sin_t(m), cos_t(m)
                STAm, STBm = sta_t(m), stb_t(m)
                ib_order = ((0, 0), (0, 1), (1, 0), (1, 1)) if m < M - 1 else \
                           ((0, 0), (1, 0), (0, 1), (1, 1))  # finish bank0 first
                for term, ib in ib_order:
                    STm = STAm if term == 0 else STBm
                    MVm = COSm if term == 0 else SINm
                    nc.tensor.matmul(acc_ps[ib][:], STm[:, ib * 128:(ib + 1) * 128],
                                     MVm[:, 256:768], start=(m == 0 and term == 0),
                                     stop=(m == M - 1 and term == 1),
                                     skip_group_check=True).then_inc(sT, 1)

        @block.vector
        def _(v):
            nc.vector.memset(halfpi[:], float(np.pi / 2))
            nc.vector.memset(magicb[:], MAGIC)
            v.wait_ge(dma_in, 16 * 9)  # awd arrives 9th
            for m in range(M):
                nc.vector.tensor_copy(AWDB[:, m * 256:(m + 1) * 256],
                                      awd_sb[:, m:m + 1].to_broadcast((128, 256)))
            v.wait_ge(sT, 2)
            nc.vector.tensor_copy(X[:, 0:256], qT_ps[:, 0:256]).then_inc(sV, 1)
            v.wait_ge(sT, 4)
            nc.vector.tensor_copy(X[:, 256:768], kT_ps[:]).then_inc(sV, 1)

            def stat_ops(j):
                g = class_gate(j)
                if g is not None:
                    v.wait_ge(sT, t_mm_end[g])
                v.wait_ge(sS, s_cos[j])
                nc.vector.tensor_tensor(sta_t(j)[:], sin_t(j)[:, 0:256],
                                        AWDB[:, j * 256:(j + 1) * 256],
                                        ALU.mult).then_inc(sV, 1)
                nc.vector.tensor_tensor(stb_t(j)[:], cos_t(j)[:, 0:256],
                                        AWDB[:, j * 256:(j + 1) * 256],
                                        ALU.mult).then_inc(sV, 1)

            for m in range(M):
                if m >= NB:
                    v.wait_ge(sS, s_cos[m - NB])  # RS/UC ring reuse
                sc = float(omegas[m] / (2 * math.pi))
                v.wait_ge(sS, s_p[m])
                nc.vector.tensor_scalar(Dt[m % 2][:], Pt[m % 4][:], -MAGIC, None,
                                        ALU.add, ALU.bypass)
                nc.vector.scalar_tensor_tensor(RS[m % NB][:], X[:], sc, Dt[m % 2][:],
                                               ALU.mult, ALU.subtract).then_inc(sV, 1)
                # |r| for the cos path: cos(2*pi*r) = sin(pi/2 - 2*pi*|r|)
                nc.vector.scalar_tensor_tensor(UC[m % NB][:], RS[m % NB][:], -1.0,
                                               RS[m % NB][:], ALU.mult,
                                               ALU.max).then_inc(sV, 1)
                if m >= 2:
                    stat_ops(m - 2)
            stat_ops(M - 2)
            stat_ops(M - 1)
            v.wait_ge(dma_in, 16 * N_DMA_IN)
            for ib in range(2):
                v.wait_ge(sT, t_mm_end[M - 1] - 2 + 2 * ib)
                nc.vector.tensor_tensor(ADD_t[ib][:], acc_ps[ib][:], gd_sb[:, ib, :],
                                        ALU.add).then_inc(sV, 1)

        @block.scalar
        def _(sc_eng):
            def p_act(j):
                nc.scalar.activation(Pt[j % 4][:], X[:], AF.Relu,
                                     scale=float(omegas[j] / (2 * math.pi)),
                                     bias=magicb[:]).then_inc(sS, 1)

            sc_eng.wait_ge(sV, 2)  # X ready
            for j in range(min(4, M)):
                p_act(j)
            for m in range(M):
                g = class_gate(m)
                if g is not None:
                    sc_eng.wait_ge(sT, t_mm_end[g])
                sc_eng.wait_ge(sV, v_rs[m])
                nc.scalar.activation(sin_t(m)[:], RS[m % NB][:], AF.Sin,
                                     scale=float(2 * math.pi)).then_inc(sS, 1)
                sc_eng.wait_ge(sV, v_uc[m])
                nc.scalar.activation(cos_t(m)[:], UC[m % NB][:], AF.Sin,
                                     scale=float(-2 * math.pi),
                                     bias=halfpi[:]).then_inc(sS, 1)
                if m + 4 <= M - 1:
                    p_act(m + 4)
            for ib in range(2):
                sc_eng.wait_ge(sV, v_add[ib])
                nc.scalar.activation(OUT_t[ib][:], ADD_t[ib][:], AF.Sigmoid,
                                     scale=float(1.0 / TAU)).then_inc(sS, 1)

        @block.sync
        def _(s):
            s.dma_start(wqT[:, 0:128], wqT_d[0]).then_inc(dma_in, 16)
            s.dma_start(wqT[:, 128:256], wqT_d[1]).then_inc(dma_in, 16)
            s.dma_start(wkT[:, 0:128], wkT_d[0]).then_inc(dma_in, 16)
            s.dma_start(wkT[:, 128:256], wkT_d[1]).then_inc(dma_in, 16)
            s.dma_start(qsT[:, 0:256], qT_d[0]).then_inc(dma_in, 16)
            s.dma_start(qsT[:, 256:512], qT_d[1]).then_inc(dma_in, 16)
            s.dma_start(ksT[:, 0:512], kT_d[0]).then_inc(dma_in, 16)
            s.dma_start(ksT[:, 512:1024], kT_d[1]).then_inc(dma_in, 16)
            s.dma_start(awd_sb[:], awd_d[:]).then_inc(dma_in, 16)
            for ib in range(2):
                s.dma_start(gd_sb[:, ib, :],
                            gd_d[ib * 128:(ib + 1) * 128, :]).then_inc(dma_in, 16)
            for ib in range(2):
                s.wait_ge(sS, s_sig[ib])
                s.dma_start(out_d[ib * 128:(ib + 1) * 128, :],
                            OUT_t[ib][:]).then_inc(dma_out, 16)
            s.wait_ge(dma_out, 32)

    return nc


_NC_CACHE = {}


def kernel(queries, keys, mask, Wq, Wk, Wv):
    from concourse.bass_utils import run_bass_kernel_spmd

    queries = np.ascontiguousarray(queries, dtype=np.float32)
    keys = np.ascontiguousarray(keys, dtype=np.float32)
    Wq = np.ascontiguousarray(Wq, dtype=np.float32)
    Wk = np.ascontiguousarray(Wk, dtype=np.float32)
    Wv = np.ascontiguousarray(Wv, dtype=np.float32)
    mask = np.asarray(mask)

    wd = (Wv[0] - Wv[1]).astype(np.float32)
    gd = (_gumbel_diff_const()
          + np.where(mask == 1, np.float32(1e6), np.float32(0.0))).astype(np.float32)

    q = queries @ Wq.T
    k = keys @ Wk.T
    xmax = float(max((q.max(axis=1) + k.max(axis=1)).max(),
                     -((q.min(axis=1) + k.min(axis=1)).min()))) + 0.05
    omegas = np.asarray(OMEGAS, dtype=np.float64)
    coef = _fit_amplitudes(omegas, max(xmax, 6.0))
    awd = (coef[None, :].astype(np.float32) * wd[:, None]).astype(np.float32)

    if "nc" not in _NC_CACHE:
        _NC_CACHE["nc"] = _build_program(OMEGAS)
    nc = _NC_CACHE["nc"]

    wqT_h = np.ascontiguousarray(Wq.T.reshape(2, 128, H))
    wkT_h = np.ascontiguousarray(Wk.T.reshape(2, 128, H))
    in_maps = []
    for c in range(NCORES):
        b, ih = c // 2, c % 2
        qT = np.ascontiguousarray(
            queries[b, ih * ROWS:(ih + 1) * ROWS, :].T.reshape(2, 128, ROWS))
        kT = np.ascontiguousarray(keys[b].T.reshape(2, 128, LK))
        in_maps.append({
            "queriesT_c": qT,
            "keysT_c": kT,
            "WqT": wqT_h, "WkT": wkT_h, "awd": awd,
            "gd_c": np.ascontiguousarray(gd[b, ih * ROWS:(ih + 1) * ROWS, :]),
        })

    res = run_bass_kernel_spmd(nc, in_maps, list(range(NCORES)))
    out = np.empty((B, LQ, LK), dtype=np.float32)
    for c in range(NCORES):
        b, ih = c // 2, c % 2
        out[b, ih * ROWS:(ih + 1) * ROWS, :] = res.results[c]["out_c"]
    return out
